# revision 6
# baseline (speedup 1.0000x reference)
"""GraphSAGE 2-layer encoder on 8 Trainium2 NeuronCores.

Strategy (dst-sharded graph parallelism):
- Nodes padded to 100352 = 784 tiles of 128; core c owns dst tiles [98c, 98c+98).
- Per layer, every core sweeps the full node-feature table (fp16) tile by tile.
  For each source tile, host-shipped fp16 selector matrices (4 "planes" + a
  leftover lane) expand the tile's rows into a DRAM grid G1[src_tile][dst_tile][plane]
  via PE matmuls; surplus edges overflow into a small round-2 grid G2.
- Each dst tile then reads its grid column with a few regular strided DMAs and
  accumulates sum_e w*h[src_e] per dst node with one-hot scatter matmuls
  (one-hots built on DVE from iota==dst_local, scaled by w=1/deg).
- Self term + aggregated term are concatenated feature-major and pushed through
  one [128x128]x[128x64] matmul against [W_l; W_r]; bias via a K=1 matmul;
  ReLU (+dropout mask from u1 for layer 1) on ACT/DVE.
- h1 is AllGather'ed (fp16) between the layers.
All floating-point compute on device is fp16 storage with fp32 PSUM accumulation.
"""

import numpy as np
import ml_dtypes

import concourse.bacc as bacc
import concourse.bass as bass
import concourse.mybir as mybir
import concourse.tile as tile
from concourse.bass_utils import run_bass_kernel_spmd

F32 = mybir.dt.float32
F16 = mybir.dt.float16
I32 = mybir.dt.int32

NCORES = 8
N = 100000
E = 1200000
D = 64
NP = 100352          # padded nodes
ST = NP // 128       # 784 source tiles
TPC = ST // NCORES   # 98 dst tiles per core
NPC = TPC * 128      # 12544 nodes per core
Q1 = 4               # grid-1 planes
LAC = 24             # leftover capacity per source tile
R1C = Q1 * TPC + LAC  # 416 selector columns
NJ2 = 147            # round-2 chunks ( = 784*24/128 )
Q2 = 6               # round-2 planes
NCELL2 = NJ2 * Q2    # 882 cells per dst tile from round 2
NB2 = 7              # round-2 scatter chunks per tile (7*128 = 896 >= 882)
JB1 = 7              # stage-B grid-1 s-blocks (7 * 112 = 784)
KB1 = 112            # stage-B grid-1 chunk K

_CACHE = {}


def _build_host_data(x, edge_index, u1, W1_l, W1_r, b1, W2_l, W2_r, b2):
    src = np.asarray(edge_index[0], np.int64)
    dst = np.asarray(edge_index[1], np.int64)
    deg = np.bincount(dst, minlength=NP).astype(np.float32)
    w_node = (1.0 / np.maximum(deg, 1.0)).astype(np.float32)

    xg = np.zeros((NP, D), np.float16)
    xg[:N] = np.asarray(x, np.float32)
    xgT = np.ascontiguousarray(xg.T)

    u1f = np.zeros((NP, D), np.float32)
    u1f[:N] = np.asarray(u1, np.float32)

    wcat1 = np.concatenate([np.asarray(W1_l), np.asarray(W1_r)], 0).astype(np.float16)
    wcat2 = np.concatenate([np.asarray(W2_l), np.asarray(W2_r)], 0).astype(np.float16)
    b1h = np.asarray(b1, np.float32).reshape(1, D).astype(np.float16)
    b2h = np.asarray(b2, np.float32).reshape(1, D).astype(np.float16)

    per_core = []
    t_glob = dst // 128
    core_of = t_glob // TPC
    for c in range(NCORES):
        m = core_of == c
        sc, dc = src[m], dst[m]
        s_t = sc // 128
        p_s = sc % 128
        t_l = t_glob[m] % TPC
        dstl = (dc % 128).astype(np.float32)
        w_e = w_node[dc]

        key = s_t * TPC + t_l
        order = np.argsort(key, kind="stable")
        ks = key[order]
        _, start, cnt = np.unique(ks, return_index=True, return_counts=True)
        rank = np.arange(len(ks)) - np.repeat(start, cnt)

        sc_o, ps_o, tl_o = s_t[order], p_s[order], t_l[order]
        dl_o, we_o = dstl[order], w_e[order]

        R1 = np.zeros((ST, 128, R1C), np.float16)
        wd1 = np.zeros((TPC, Q1, JB1, KB1, 2), np.float32)
        R2 = np.zeros((NJ2, 128, Q2 * TPC), np.float16)
        wd2 = np.zeros((TPC, NB2, 128, 2), np.float32)

        mn = rank < Q1
        q = rank[mn]
        s1, p1, t1 = sc_o[mn], ps_o[mn], tl_o[mn]
        R1[s1, p1, q * TPC + t1] = 1.0
        wd1[t1, q, s1 // KB1, s1 % KB1, 0] = dl_o[mn]
        wd1[t1, q, s1 // KB1, s1 % KB1, 1] = we_o[mn]

        ml_ = rank >= Q1
        ls, lp, lt = sc_o[ml_], ps_o[ml_], tl_o[ml_]
        ldl, lwe = dl_o[ml_], we_o[ml_]
        o2 = np.argsort(ls, kind="stable")
        ls2, lp2, lt2 = ls[o2], lp[o2], lt[o2]
        ldl2, lwe2 = ldl[o2], lwe[o2]
        _, st2, cn2 = np.unique(ls2, return_index=True, return_counts=True)
        l_idx = np.arange(len(ls2)) - np.repeat(st2, cn2)
        if len(cn2) and cn2.max() > LAC:
            raise RuntimeError(f"leftover cap exceeded: {cn2.max()}")
        R1[ls2, lp2, Q1 * TPC + l_idx] = 1.0
        slot = ls2 * LAC + l_idx
        j2 = slot % NJ2
        p2 = slot // NJ2
        key2 = j2 * TPC + lt2
        o3 = np.argsort(key2, kind="stable")
        k2s = key2[o3]
        _, st3, cn3 = np.unique(k2s, return_index=True, return_counts=True)
        rank2 = np.arange(len(k2s)) - np.repeat(st3, cn3)
        if len(cn3) and cn3.max() > Q2:
            raise RuntimeError(f"round-2 cap exceeded: {cn3.max()}")
        j2o, p2o, t2o = j2[o3], p2[o3], lt2[o3]
        R2[j2o, p2o, rank2 * TPC + t2o] = 1.0
        cell = j2o * Q2 + rank2
        wd2[t2o, cell // 128, cell % 128, 0] = ldl2[o3]
        wd2[t2o, cell // 128, cell % 128, 1] = lwe2[o3]

        per_core.append(
            dict(
                R1=R1.view(np.uint16),
                R2=R2.view(np.uint16),
                wd1=wd1,
                wd2=wd2,
                u1s=u1f[c * NPC : (c + 1) * NPC],
                xg=xg.view(np.uint16),
                xgT=np.ascontiguousarray(xgT[:, c * NPC : (c + 1) * NPC]).view(np.uint16),
                wcat1=wcat1.view(np.uint16),
                wcat2=wcat2.view(np.uint16),
                b1=b1h.view(np.uint16),
                b2=b2h.view(np.uint16),
            )
        )
    return per_core


def _build_program(with_collective=True):
    nc = bacc.Bacc("TRN2", target_bir_lowering=False, debug=False, num_devices=NCORES)
    xg = nc.dram_tensor("xg", [NP, D], F16, kind="ExternalInput")
    xgT = nc.dram_tensor("xgT", [D, NPC], F16, kind="ExternalInput")
    R1 = nc.dram_tensor("R1", [ST, 128, R1C], F16, kind="ExternalInput")
    R2 = nc.dram_tensor("R2", [NJ2, 128, Q2 * TPC], F16, kind="ExternalInput")
    wd1 = nc.dram_tensor("wd1", [TPC, Q1, JB1, KB1, 2], F32, kind="ExternalInput")
    wd2 = nc.dram_tensor("wd2", [TPC, NB2, 128, 2], F32, kind="ExternalInput")
    u1s = nc.dram_tensor("u1s", [NPC, D], F32, kind="ExternalInput")
    wcat1 = nc.dram_tensor("wcat1", [128, D], F16, kind="ExternalInput")
    wcat2 = nc.dram_tensor("wcat2", [128, D], F16, kind="ExternalInput")
    b1 = nc.dram_tensor("b1", [1, D], F16, kind="ExternalInput")
    b2 = nc.dram_tensor("b2", [1, D], F16, kind="ExternalInput")
    out = nc.dram_tensor("out", [NPC, D], F32, kind="ExternalOutput")

    with tile.TileContext(nc) as tc:
        with (
            tc.tile_pool(name="const", bufs=1) as cpool,
            tc.tile_pool(name="dram", bufs=1, space="DRAM") as dpool,
        ):
            G1 = dpool.tile([ST, TPC, Q1 * D], F16)
            LA = dpool.tile([ST * LAC, D], F16)
            G2L = dpool.tile([TPC, NB2 * 128, D], F16)
            h1_local = dpool.tile([NPC, D], F16)
            h1g = dpool.tile([NP, D], F16)

            iota_i = cpool.tile([128, 128], I32)
            nc.gpsimd.iota(iota_i[:], pattern=[[1, 128]], base=0, channel_multiplier=0)
            iota_f = cpool.tile([128, 128], F32)
            nc.vector.tensor_copy(iota_f[:], iota_i[:])
            ident = cpool.tile([128, 128], F32)
            nc.gpsimd.memset(ident[:], 0.0)
            nc.gpsimd.affine_select(
                out=ident[:], in_=ident[:], compare_op=mybir.AluOpType.not_equal,
                fill=1.0, base=0, pattern=[[-1, 128]], channel_multiplier=1,
            )
            ones1 = cpool.tile([1, 128], F16)
            nc.vector.memset(ones1[:], 1.0)
            wc_sb = [cpool.tile([128, D], F16, tag=f"wc{i}", name=f"wc{i}") for i in range(2)]
            nc.sync.dma_start(wc_sb[0][:], wcat1[:, :])
            nc.sync.dma_start(wc_sb[1][:], wcat2[:, :])
            b_sb = [cpool.tile([1, D], F16, tag=f"b{i}", name=f"bb{i}") for i in range(2)]
            nc.sync.dma_start(b_sb[0][:], b1[:, :])
            nc.sync.dma_start(b_sb[1][:], b2[:, :])

            # zero-fill LA (pad slots) and G2L pad rows
            zt = cpool.tile([128, NJ2 * D], F16)
            nc.vector.memset(zt[:], 0.0)
            nc.sync.dma_start(
                LA[:].rearrange("(a p) f -> p a f", p=128), zt[:].rearrange("p (a f) -> p a f", f=D)
            )
            npad2 = NB2 * 128 - NCELL2
            nc.sync.dma_start(
                G2L[:, NCELL2:, :].rearrange("t a f -> t (a f)"),
                zt[:TPC, : npad2 * D],
            )

            for L in range(2):
                srct = xg if L == 0 else h1g
                # ---- stage A: expansion sweep ----
                with (
                    tc.tile_pool(name=f"sa{L}", bufs=3) as pa,
                    tc.tile_pool(name=f"psa{L}", bufs=2, space="PSUM") as ppa,
                ):
                    for s in range(ST):
                        if s % 4 == 0:
                            xt4 = pa.tile([128, 4 * D], F16, tag="xt4")
                            nc.sync.dma_start(
                                xt4[:].rearrange("p (b f) -> p b f", f=D),
                                srct[s * 128 : (s + 4) * 128, :].rearrange(
                                    "(b p) f -> p b f", p=128
                                ),
                            )
                        xt = xt4[:, (s % 4) * D : (s % 4 + 1) * D]
                        rt = pa.tile([128, R1C], F16, tag="rt")
                        nc.sync.dma_start(rt[:], R1[s])
                        psA = ppa.tile([TPC, Q1 * D], F32, tag="psA", space="PSUM")
                        for q in range(Q1):
                            nc.tensor.matmul(
                                psA[:, q * D : (q + 1) * D],
                                lhsT=rt[:, q * TPC : (q + 1) * TPC],
                                rhs=xt, start=True, stop=True,
                            )
                        psL = ppa.tile([LAC, D], F32, tag="psL", space="PSUM")
                        nc.tensor.matmul(
                            psL[:], lhsT=rt[:, Q1 * TPC : R1C], rhs=xt,
                            start=True, stop=True,
                        )
                        sbA = pa.tile([TPC, Q1 * D], F16, tag="sbA")
                        nc.vector.tensor_copy(sbA[:], psA[:])
                        nc.sync.dma_start(G1[s], sbA[:])
                        sbL = pa.tile([LAC, D], F16, tag="sbL")
                        nc.vector.tensor_copy(sbL[:], psL[:])
                        nc.sync.dma_start(LA[s * LAC : (s + 1) * LAC, :], sbL[:])

                # ---- round 2: leftover routing ----
                with (
                    tc.tile_pool(name=f"r2{L}", bufs=3) as pr,
                    tc.tile_pool(name=f"psr{L}", bufs=2, space="PSUM") as ppr,
                ):
                    for j in range(NJ2):
                        lt_ = pr.tile([128, D], F16, tag="lt")
                        nc.sync.dma_start(
                            lt_[:], LA[:].rearrange("(p a) f -> p a f", a=NJ2)[:, j, :]
                            if False
                            else LA[:].rearrange("(a p) f -> a p f", a=128)[:, j, :],
                        )
                        r2t = pr.tile([128, Q2 * TPC], F16, tag="r2t")
                        nc.sync.dma_start(r2t[:], R2[j])
                        ps2 = ppr.tile([TPC, Q2 * D], F32, tag="ps2", space="PSUM")
                        for q in range(Q2):
                            nc.tensor.matmul(
                                ps2[:, q * D : (q + 1) * D],
                                lhsT=r2t[:, q * TPC : (q + 1) * TPC],
                                rhs=lt_[:], start=True, stop=True,
                            )
                        sb2 = pr.tile([TPC, Q2 * D], F16, tag="sb2")
                        nc.vector.tensor_copy(sb2[:], ps2[:])
                        nc.sync.dma_start(
                            G2L[:, j * Q2 : (j + 1) * Q2, :].rearrange("t a f -> t (a f)"),
                            sb2[:],
                        )

                # ---- stage B: per-dst-tile scatter + dense epilogue ----
                with (
                    tc.tile_pool(name=f"sb{L}", bufs=3) as pb,
                    tc.tile_pool(name=f"psb{L}", bufs=2, space="PSUM") as ppb,
                ):
                    for t in range(TPC):
                        g1t = pb.tile([KB1, JB1 * Q1 * D], F16, tag="g1t")
                        nc.sync.dma_start(
                            g1t[:].rearrange("p (j q f) -> p j q f", q=Q1, f=D),
                            G1[:, t, :]
                            .rearrange("(j p) (q f) -> p j q f", p=KB1, f=D),
                        )
                        g2t = pb.tile([128, NB2 * D], F16, tag="g2t")
                        nc.sync.dma_start(
                            g2t[:].rearrange("p (b f) -> p b f", f=D),
                            G2L[t].rearrange("(b p) f -> p b f", p=128),
                        )
                        w1t = pb.tile([KB1, Q1 * JB1 * 2], F32, tag="w1t")
                        nc.sync.dma_start(
                            w1t[:].rearrange("p (q j c) -> p q j c", q=Q1, c=2),
                            wd1[t].rearrange("q j p c -> p q j c"),
                        )
                        w2t = pb.tile([128, NB2 * 2], F32, tag="w2t")
                        nc.sync.dma_start(
                            w2t[:].rearrange("p (b c) -> p b c", c=2),
                            wd2[t].rearrange("b p c -> p b c"),
                        )
                        pagg = ppb.tile([D, 128], F32, tag="pagg", space="PSUM")
                        nmm = Q1 * JB1 + NB2
                        mi = 0
                        for q in range(Q1):
                            for j in range(JB1):
                                oh = pb.tile([KB1, 128], F16, tag="oh")
                                col = (q * JB1 + j) * 2
                                nc.vector.tensor_scalar(
                                    out=oh[:], in0=iota_f[:KB1, :],
                                    scalar1=w1t[:, col : col + 1],
                                    scalar2=w1t[:, col + 1 : col + 2],
                                    op0=mybir.AluOpType.is_equal,
                                    op1=mybir.AluOpType.mult,
                                )
                                nc.tensor.matmul(
                                    pagg[:],
                                    lhsT=g1t[:, (j * Q1 + q) * D : (j * Q1 + q + 1) * D],
                                    rhs=oh[:], start=(mi == 0), stop=(mi == nmm - 1),
                                )
                                mi += 1
                        for bq in range(NB2):
                            oh2 = pb.tile([128, 128], F16, tag="oh2")
                            nc.vector.tensor_scalar(
                                out=oh2[:], in0=iota_f[:],
                                scalar1=w2t[:, 2 * bq : 2 * bq + 1],
                                scalar2=w2t[:, 2 * bq + 1 : 2 * bq + 2],
                                op0=mybir.AluOpType.is_equal,
                                op1=mybir.AluOpType.mult,
                            )
                            nc.tensor.matmul(
                                pagg[:], lhsT=g2t[:, bq * D : (bq + 1) * D],
                                rhs=oh2[:], start=(mi == 0), stop=(mi == nmm - 1),
                            )
                            mi += 1
                        cat = pb.tile([128, 128], F16, tag="cat")
                        nc.vector.tensor_copy(cat[:D, :], pagg[:])
                        if L == 0:
                            nc.sync.dma_start(
                                cat[D:128, :], xgT[:, t * 128 : (t + 1) * 128]
                            )
                        else:
                            ht = pb.tile([128, D], F16, tag="ht")
                            nc.sync.dma_start(ht[:], h1_local[t * 128 : (t + 1) * 128, :])
                            htf = pb.tile([128, D], F32, tag="htf")
                            nc.vector.tensor_copy(htf[:], ht[:])
                            pst = ppb.tile([D, 128], F32, tag="pst", space="PSUM")
                            nc.tensor.transpose(pst[:], htf[:], ident[:])
                            nc.vector.tensor_copy(cat[D:128, :], pst[:])
                        pout = ppb.tile([128, D], F32, tag="pout", space="PSUM")
                        nc.tensor.matmul(pout[:], lhsT=cat[:], rhs=wc_sb[L][:], start=True, stop=False)
                        nc.tensor.matmul(pout[:], lhsT=ones1[:], rhs=b_sb[L][:], start=False, stop=True)
                        if L == 0:
                            h16 = pb.tile([128, D], F16, tag="h16")
                            nc.scalar.activation(
                                h16[:], pout[:], mybir.ActivationFunctionType.Relu, scale=2.0
                            )
                            u1t = pb.tile([128, D], F32, tag="u1t")
                            nc.sync.dma_start(u1t[:], u1s[t * 128 : (t + 1) * 128, :])
                            msk = pb.tile([128, D], F16, tag="msk")
                            nc.vector.tensor_scalar(
                                out=msk[:], in0=u1t[:], scalar1=0.5, scalar2=None,
                                op0=mybir.AluOpType.is_gt,
                            )
                            h1t = pb.tile([128, D], F16, tag="h1t")
                            nc.vector.tensor_tensor(
                                h1t[:], h16[:], msk[:], op=mybir.AluOpType.mult
                            )
                            nc.sync.dma_start(h1_local[t * 128 : (t + 1) * 128, :], h1t[:])
                        else:
                            of = pb.tile([128, D], F32, tag="of")
                            nc.scalar.activation(
                                of[:], pout[:], mybir.ActivationFunctionType.Relu
                            )
                            nc.sync.dma_start(out[t * 128 : (t + 1) * 128, :], of[:])

                if L == 0 and with_collective:
                    nc.gpsimd.collective_compute(
                        "AllGather", mybir.AluOpType.bypass,
                        replica_groups=[list(range(NCORES))],
                        ins=[h1_local[:].opt()], outs=[h1g[:].opt()],
                    )
    nc.compile()
    return nc


def kernel(**inputs):
    x = np.asarray(inputs["x"], np.float32)
    edge_index = np.asarray(inputs["edge_index"])
    u1 = np.asarray(inputs["u1"], np.float32)
    per_core = _build_host_data(
        x, edge_index, u1,
        inputs["W1_l"], inputs["W1_r"], inputs["b1"],
        inputs["W2_l"], inputs["W2_r"], inputs["b2"],
    )
    if "nc" not in _CACHE:
        _CACHE["nc"] = _build_program()
    nc = _CACHE["nc"]
    res = run_bass_kernel_spmd(nc, per_core, core_ids=list(range(NCORES)))
    outs = [np.asarray(res.results[c]["out"], np.float32) for c in range(NCORES)]
    full = np.concatenate(outs, axis=0)
    return full[:N]
